# revision 25
# baseline (speedup 1.0000x reference)
"""Trainium2 Bass kernel for nn_GCNSampling (gnn_message_passing).

Computation:
    h0  = relu(features @ W1.T + b1)        # [N0, 128]
    h1  = h0[map1]                          # [N1, 128]
    agg = mean(h1[neigh_idx], axis=1)       # [N2, 128]
    out = agg @ W2.T + b2                   # [N2, 41]

Strategy (seed-sharded, gather-free, fp8 features):
  The two gather levels are folded on the host: idx2 = map1[neigh_idx] maps
  every (seed, neighbor) slot directly to a layer-0 node. The host expands
  features to slot order (features[idx2], 25 rows per seed, seed-major),
  quantizes to fp8 e3m4 (x2 scale; relu is positively homogeneous so the
  scale folds into W2), and pre-transposes so the device runs a dense
  fused pass per core:

      featT[:, slot] --5 matmuls (602 = 4x128 + 90), W1 bf16--> PSUM
          --ScalarE relu(+b1)--> h0 bf16 [128, 500]
          --DVE contiguous reduce over each seed's 25 slots--> acc f32r
      acc[128, seeds] --f32r matmul W2--> logitsT --(+b2)--> y [48, seeds]

  Group width 500 = 20 seeds x fan 25, so every group reduces with a
  stride-1 inner dim straight into its acc columns, and matmul cost is
  dominated by per-instruction overhead, so wide groups win.  fp8
  halves feature DMA vs bf16; the matmuls run at bf16 speed (1 col/
  cycle).  Logits for each finished 512-seed segment are emitted inside
  the main loop (no serial epilogue).

  DMA model (measured): each line (contiguous run) is served by one of
  16 engines at ~22.5 GB/s with ~85ns per-descriptor overhead, so
  chunk_time ~ ceil(lines/16) * (line_bytes/22.5G + 85ns).  Features
  are packed so slabs 0-3 of each partition are adjacent in DRAM
  (quad lines of 4*cw bytes, 128+90 lines per chunk instead of 602).
  Chunks ramp 1000/2000/4000 then 6000 so compute starts ~12us in and
  never starves after; ramp chunks zero-pad the 90-row tail slab to 128
  rows so each is a single one-trigger transfer, and chunk 0 rides in
  the SAME uint8 DMA as the w1 weights (bitcast views) — one trigger
  opens the whole pipeline and the LDWEIGHTS dependency is safe by
  construction (single tile, single writer).

  Measured dead ends on this platform (kept for posterity):
  - perf_mode=DoubleRow (and by extension MX) is faster per instruction
    but trips the chip's power throttle (util capped at 50% for half
    the run) for zero net win; the PE multiply rate is the power wall.
    The matmul stream (5 passes x 31250 cols at 1 col/cycle) is the
    ~72us floor.
  - Splitting feature DMA across two trigger-engine queues halves each
    queue's rate (~130 GB/s each vs ~290 single) - shared backend.
  - Per-partition mega-lines (60KB) starve parallelism: only 128 lines
    across 16 engines at 22.5 GB/s each. 8-24KB lines are the sweet
    spot.
  - Dummy p-state warm-up matmuls just delay the real stream.
  - Loading w1t from a non-sync queue races LDWEIGHTS (cross-queue dep
    is only implied by same-queue FIFO order ahead of the first chunk).
  - fp8 h0 storage: no time win (ScalarE/DVE are element-rate bound,
    not byte bound), 2x the output error. bf16 h0 kept.
  Device-side gathers are avoided entirely (SWDGE descriptor generation
  is too slow for ~31k gathered rows/core).

  Sharding: seeds split evenly across 8 cores (1250 seeds each, no
  padding), weights replicated, no collectives. Accuracy: e3m4 features
  + bf16 W1 give rel err ~5.8e-3 (vs 2e-2 budget) — verified
  deterministically in numpy and CoreSim against this problem's fixed
  inputs.  Measured: 89.0-92.1us, median ~90 (bf16 predecessor:
  143.7us); remaining time = 5.8us NEFF init + ~6us first-chunk
  latency + the 72us matmul floor + ~2us stalls/drain.  Ramp-shape
  variants trade start latency against mid-stalls ~1:1 (supply rate
  during ramp ~= demand rate); this shape minimizes the worst case.
"""

import math

import numpy as np
import ml_dtypes

import concourse.bacc as bacc
import concourse.mybir as mybir
import concourse.tile as tile
from concourse import bass_utils

N_CORES = 8
HIDDEN = 128
CPAD = 48  # classes padded 41 -> 48
SF = 2.0        # feature quantization scale
WS = 64.0       # W1 scale for e4m3 (avoids subnormals; power of 2 = exact)
F8 = ml_dtypes.float8_e3m4   # TRN fp8e3 (tail slab)
F8E4 = ml_dtypes.float8_e4m3  # TRN fp8e4 (DoubleRow slabs)
GW = 500        # group width: 20 seeds x fan 25
DR_K = 256      # contraction rows per DoubleRow pass

# Set by test harness: run with trace=True and record exec time here.
TRACE = False
SIM = False
LAST_EXEC_NS = None
LAST_RES = None

_BUILD_CACHE = {}


def _slabs(n_feats):
    slabs = []
    o = 0
    while o + 128 <= n_feats:
        slabs.append((o, 128))
        o += 128
    if o < n_feats:
        slabs.append((o, n_feats - o))
    return slabs


def _chunk_schedule(slots, chunk):
    """chunk0 = GW (rides with w1 for an early compute start); then full
    `chunk`s at peak DMA line size (24KB quad lines); then a geometric
    ramp-DOWN tail. Compute per chunk (~1.62ns/slot) is barely below DMA
    (~1.7ns/slot + fixed), so a big final chunk would serialize ~10us of
    compute after the last DMA byte — the ramp-down nests the compute
    tail inside the DMA tail instead."""
    c0w = 250  # chunk0 rides with w1; small = earlier first matmul
    tail = slots % GW
    rem = slots - tail - c0w
    chunks = [c0w]
    down = [3000, 1500, 750, 500]
    while rem > chunk + sum(down):
        chunks.append(min(chunk, rem - sum(down)))
        rem -= chunks[-1]
    if rem > sum(down):
        chunks.append(rem - sum(down))
        rem = sum(down)
    while down and rem:
        take = min(down.pop(0), rem)
        chunks.append(take)
        rem -= take
    if tail:
        chunks[-1] += tail
    return chunks


def _build(n_feats, nseed, fan, chunk):
    """Build + compile the per-core program (identical on all 8 cores)."""
    F32 = mybir.dt.float32
    F32R = mybir.dt.float32r
    DT_DR = mybir.dt.float8e4   # DoubleRow slabs: features + W1
    DT_TAIL = mybir.dt.float8e3  # tail-slab features
    DT_W = mybir.dt.bfloat16     # tail-slab W1
    DT_H = mybir.dt.bfloat16

    slots = nseed * fan
    assert GW % fan == 0 and chunk % GW == 0

    slabs = _slabs(n_feats)
    ns = len(slabs)
    nfull = sum(1 for _, kk in slabs if kk == 128)
    assert nfull % 2 == 0, "DoubleRow pairing wants an even full-slab count"
    npairs = nfull // 2
    has_tail = ns > nfull

    nc = bacc.Bacc("TRN2", target_bir_lowering=False, debug=False,
                   num_devices=N_CORES)
    chunks = _chunk_schedule(slots, chunk)
    # chunk 0 rides in one DMA with the w1 weights (uint8 tile, bitcast
    # views); later chunks are packed features, ramp ones zero-padded to
    # 128 rows per slab
    # w1 bytes/partition: per DR pass 2 k-tiles x 128 h x 1B, tail 128 h bf16
    W1B = npairs * 2 * HIDDEN + (2 * HIDDEN if has_tail else 0)
    cw0 = chunks[0]
    featT_len = sum(n_feats * cw for cw in chunks[1:])
    featT = nc.dram_tensor("featT", [featT_len], DT_DR,
                           kind="ExternalInput").ap()
    wf0 = nc.dram_tensor("wf0", [128, W1B + ns * cw0], mybir.dt.uint8,
                         kind="ExternalInput").ap()
    w2t = nc.dram_tensor("w2t", [HIDDEN, CPAD], F32R,
                         kind="ExternalInput").ap()
    b1 = nc.dram_tensor("b1", [128, 1], F32, kind="ExternalInput").ap()
    b2 = nc.dram_tensor("b2", [CPAD, 1], F32, kind="ExternalInput").ap()
    # transposed output: yT[c, seed]; host transposes back (tiny)
    y = nc.dram_tensor("y", [CPAD, nseed], F32, kind="ExternalOutput").ap()

    with tile.TileContext(nc) as tc:
        with (
            # f32r accumulator (fp22-rounded, rel 1e-4) feeds the final
            # f32r matmul; quantization noise dominates by 100x
            nc.allow_low_precision(reason="f32r acc for final matmul"),
            tc.tile_pool(name="const", bufs=1) as const,
            tc.tile_pool(name="feat", bufs=3) as featp,
            tc.tile_pool(name="out", bufs=3) as outp,
            tc.tile_pool(name="ph", bufs=4, space="PSUM") as php,
            tc.tile_pool(name="pa", bufs=2, space="PSUM") as pap,
        ):
            # w1 weights + chunk-0 features in ONE transfer on the sync
            # queue (single trigger; LDWEIGHTS and the first matmul both
            # depend on this one tile); other constants via scalar queue
            combo = const.tile([128, W1B + ns * cw0], mybir.dt.uint8)
            nc.sync.dma_start(combo[:], wf0[:])

            def w1_dr_ap(j):
                # [128, 2, 128] e4m3: partition p holds W1T rows
                # (256j + p, 256j + 128 + p) for the two k-tiles
                return combo[:, j * 2 * HIDDEN:(j + 1) * 2 * HIDDEN] \
                    .bitcast(DT_DR).rearrange("p (i h) -> p i h", i=2)

            def w1_tail_ap(kk):
                o = npairs * 2 * HIDDEN
                return combo[:kk, o:o + 2 * HIDDEN].bitcast(DT_W)

            def fk0_dr(j, a, b):
                base = W1B + 2 * j * cw0
                return combo[:, base:base + 2 * cw0].bitcast(DT_DR) \
                    .rearrange("p (s c) -> p s c", s=2)[:, :, a:b]

            def fk0_tail(kk, a, b):
                o = W1B + nfull * cw0
                return combo[:kk, o + a:o + b].bitcast(DT_TAIL)

            b1_sb = const.tile([128, 1], F32)
            nc.scalar.dma_start(b1_sb[:], b1[:])
            w2t_sb = const.tile([128, CPAD], F32R)
            nc.scalar.dma_start(w2t_sb[:], w2t[:])
            b2_sb = const.tile([CPAD, 1], F32)
            nc.scalar.dma_start(b2_sb[:], b2[:])
            # per-seed accumulator over the fan, in h0 space
            acc = const.tile([128, nseed], F32R)

            c0 = 0
            off = 0
            seg0 = 0
            for ci, cw in enumerate(chunks):
                # quad-packed lines: 4 slabs adjacent per partition in
                # DRAM (each DMA line runs on one engine at ~22.5GB/s;
                # fewer, bigger lines win until ~16KB)
                if ci == 0:
                    fk = None
                else:
                    fk = featp.tile([128, ns * cw], DT_DR, tag="fk")
                    step = 4 if nfull % 4 == 0 else 2
                    for j in range(0, nfull, step):
                        nc.sync.dma_start(
                            fk[:, j * cw:(j + step) * cw],
                            featT[off:off + step * 128 * cw].rearrange(
                                "(p c) -> p c", p=128),
                        )
                        off += step * 128 * cw
                    if ns > nfull:
                        kk = slabs[nfull][1]
                        nc.sync.dma_start(
                            fk[:kk, nfull * cw:ns * cw],
                            featT[off:off + kk * cw].rearrange(
                                "(p c) -> p c", p=kk),
                        )
                        off += kk * cw

                for g0 in range(0, cw, GW):
                    gw = min(GW, cw - g0)
                    ph = php.tile([128, GW], F32, tag="ph", space="PSUM")
                    for j in range(npairs):
                        rhs = (fk0_dr(j, g0, g0 + gw) if ci == 0 else
                               fk[:, 2 * j * cw:(2 * j + 2) * cw]
                               .rearrange("p (s c) -> p s c", s=2)
                               [:, :, g0:g0 + gw])
                        nc.tensor.matmul(
                            ph[:, :gw],
                            w1_dr_ap(j),
                            rhs,
                            start=(j == 0),
                            stop=(not has_tail and j == npairs - 1),
                            perf_mode=mybir.MatmulPerfMode.DoubleRow,
                        )
                    if has_tail:
                        kk = slabs[nfull][1]
                        rhs = (fk0_tail(kk, g0, g0 + gw) if ci == 0 else
                               fk[:kk, nfull * cw + g0:nfull * cw + g0 + gw]
                               .bitcast(DT_TAIL))
                        nc.tensor.matmul(
                            ph[:, :gw],
                            w1_tail_ap(kk),
                            rhs,
                            start=False,
                            stop=True,
                        )
                    # relu(+b1) IN PLACE in PSUM, then reduce straight from
                    # PSUM: drops the h0 SBUF tile and its buffer-free
                    # dependency edges (each cost a ~600ns scalar event-sem)
                    nc.scalar.activation(ph[:, :gw], ph[:, :gw],
                                         mybir.ActivationFunctionType.Relu,
                                         bias=b1_sb[:, 0:1])
                    # cols are seed-major (fan inner): one contiguous
                    # reduce per group straight into acc columns
                    s_base = (c0 + g0) // fan
                    nsd = gw // fan
                    nc.vector.reduce_sum(
                        acc[:, s_base:s_base + nsd],
                        ph[:, :gw].rearrange("h (s r) -> h s r", r=fan),
                        axis=mybir.AxisListType.X)
                    # emit logits for every fully-aggregated 512-seed
                    # segment so the epilogue isn't one serial tail.
                    # Trigger 2 groups late: the pa matmul waits on the
                    # segment's last DVE reduce, and the in-order PE queue
                    # would stall behind it if emitted at the boundary.
                    done = s_base + nsd
                    flush = (c0 + g0 + gw == slots)
                    while seg0 < nseed:
                        seg_end = min(seg0 + 512, nseed)
                        if seg_end == nseed and seg_end - seg0 > 64:
                            # split the final segment so only a 64-seed
                            # sliver trails the last group (shorter
                            # reduce->matmul->identity->DMA drain chain)
                            seg_end = nseed - 64
                        if not flush and done < seg_end + 40:
                            break
                        sw = seg_end - seg0
                        pa = pap.tile([CPAD, 512], F32, tag="pa",
                                      space="PSUM")
                        nc.tensor.matmul(pa[:, :sw], w2t_sb[:],
                                         acc[:, seg0:seg0 + sw],
                                         start=True, stop=True)
                        yo = outp.tile([CPAD, 512], F32, tag="yo")
                        nc.scalar.activation(
                            yo[:, :sw], pa[:, :sw],
                            mybir.ActivationFunctionType.Identity,
                            bias=b2_sb[:, 0:1])
                        nc.sync.dma_start(y[:, seg0:seg0 + sw],
                                          yo[:, :sw])
                        seg0 += sw
                c0 += cw

    nc.compile()
    return nc


def kernel(features, W1, b1, W2, b2, map1, neigh_idx):
    global LAST_EXEC_NS, LAST_RES
    features = np.asarray(features, dtype=np.float32)
    W1 = np.asarray(W1, dtype=np.float32)
    b1 = np.asarray(b1, dtype=np.float32)
    W2 = np.asarray(W2, dtype=np.float32)
    b2 = np.asarray(b2, dtype=np.float32)
    map1 = np.asarray(map1).astype(np.int64)
    neigh_idx = np.asarray(neigh_idx).astype(np.int64)

    n0, n_feats = features.shape
    hidden = W1.shape[0]
    classes = W2.shape[0]
    n2, fan = neigh_idx.shape
    assert hidden == HIDDEN and classes <= CPAD

    idx2 = map1[neigh_idx]  # [N2, fan] -> layer-0 node per slot

    # split seeds evenly; pad only to a multiple of N_CORES
    spc = math.ceil(n2 / N_CORES)  # seeds per core
    n2_pad = spc * N_CORES
    if n2_pad > n2:
        idx2 = np.concatenate(
            [idx2, np.zeros((n2_pad - n2, fan), dtype=idx2.dtype)], axis=0)

    chunk = 8000
    nc = _get_built(n_feats, spc, fan, chunk)
    slots = spc * fan
    chunks = _chunk_schedule(slots, chunk)
    slabs = _slabs(n_feats)

    # quantize once, gather bytes per slot (cheap on host).
    # DR slabs (features rows [0, 128*nfull)): e4m3 * SF; tail rows: e3m4 * SF
    nfull_h = sum(1 for _, kk in slabs if kk == 128)
    npairs_h = nfull_h // 2
    kdr = 128 * nfull_h
    fq = np.empty((n0, n_feats), dtype=np.uint8)
    fq[:, :kdr] = np.asarray(features[:, :kdr] * SF, F8E4).view(np.uint8)
    fq[:, kdr:] = np.asarray(features[:, kdr:] * SF, F8).view(np.uint8)
    # W1 scaled by WS (exact power of two): DR passes e4m3, tail bf16
    w1s = (W1.T * WS).astype(np.float32)              # [F, 128]
    W1B = npairs_h * 2 * HIDDEN + (2 * HIDDEN if n_feats > kdr else 0)
    w1q_u8 = np.zeros((128, W1B), dtype=np.uint8)
    for j in range(npairs_h):
        for i in range(2):
            o = 256 * j + 128 * i
            blk = np.asarray(w1s[o:o + 128], F8E4)    # [128 part, 128 h]
            w1q_u8[:, (2 * j + i) * HIDDEN:(2 * j + i + 1) * HIDDEN] = \
                blk.view(np.uint8)
    if n_feats > kdr:
        tl = np.zeros((128, HIDDEN), dtype=np.float32)
        tl[:n_feats - kdr] = w1s[kdr:]
        w1q_u8[:, npairs_h * 2 * HIDDEN:] = \
            tl.astype(ml_dtypes.bfloat16).view(np.uint8)
    b1_in = np.ascontiguousarray((b1 * SF * WS).reshape(HIDDEN, 1)).astype(
        np.float32)
    w2t = np.zeros((HIDDEN, CPAD), dtype=np.float32)
    w2t[:, :classes] = (W2 / (SF * WS * fan)).T
    b2_in = np.zeros((CPAD, 1), dtype=np.float32)
    b2_in[:classes, 0] = b2

    in_maps = []
    for c in range(N_CORES):
        slot_ids = idx2[c * spc:(c + 1) * spc].ravel()  # seed-major
        featT = np.ascontiguousarray(fq[slot_ids].T)  # [F, slots] fp8 bytes
        # pack slab-PAIR-interleaved per chunk: partition p's rows for
        # slabs (2j, 2j+1) are adjacent -> 2*cw-byte DMA lines
        parts = []
        c0 = 0
        wf0 = None
        for ci, cw in enumerate(chunks):
            if ci == 0:
                ns_h = len(slabs)
                blk = np.zeros((128, ns_h, cw), dtype=featT.dtype)
                for i, (o, kk) in enumerate(slabs):
                    blk[:kk, i] = featT[o:o + kk, c0:c0 + cw]
                wf0 = np.concatenate(
                    [w1q_u8, blk.reshape(128, -1).view(np.uint8)], axis=1)
            else:
                step = 4 if nfull_h % 4 == 0 else 2
                for j in range(0, nfull_h, step):
                    blk = featT[j * 128:(j + step) * 128, c0:c0 + cw]
                    parts.append(
                        blk.reshape(step, 128, cw).transpose(1, 0, 2)
                        .ravel())
                if len(slabs) > nfull_h:
                    o, kk = slabs[nfull_h]
                    parts.append(featT[o:o + kk, c0:c0 + cw].ravel())
            c0 += cw
        featT_packed = np.concatenate(parts).view(F8E4)
        in_maps.append({
            "featT": featT_packed,
            "wf0": wf0,
            "w2t": w2t,
            "b1": b1_in,
            "b2": b2_in,
        })

    if SIM:
        from concourse.bass_interp import CoreSim

        ys = []
        for c in range(N_CORES if SIM is True else int(SIM)):
            sim = CoreSim(nc, trace=False)
            for k, v in in_maps[c].items():
                sim.tensor(k)[:] = v
            sim.simulate(check_with_hw=False)
            ys.append(sim.tensor("y").T.copy())
        LAST_EXEC_NS = None
        yf = np.concatenate(ys, axis=0)
        if len(ys) < N_CORES:  # partial sim: pad to full shape with zeros
            yf = np.concatenate(
                [yf, np.zeros((n2_pad - yf.shape[0], CPAD), yf.dtype)], axis=0)
    else:
        res = bass_utils.run_bass_kernel_spmd(
            nc, in_maps, core_ids=list(range(N_CORES)), trace=TRACE)
        LAST_EXEC_NS = res.exec_time_ns
        LAST_RES = res
        yf = np.concatenate(
            [res.results[c]["y"].T for c in range(N_CORES)], axis=0)
    return np.ascontiguousarray(yf[:n2, :classes]).astype(np.float32)


def _get_built(n_feats, nseed, fan, chunk):
    key = (n_feats, nseed, fan, chunk)
    if key not in _BUILD_CACHE:
        _BUILD_CACHE[key] = _build(n_feats, nseed, fan, chunk)
    return _BUILD_CACHE[key]



# revision 27
# speedup vs baseline: 1.1117x; 1.1117x over previous
"""Trainium2 Bass kernel for nn_GCNSampling (gnn_message_passing).

Computation:
    h0  = relu(features @ W1.T + b1)        # [N0, 128]
    h1  = h0[map1]                          # [N1, 128]
    agg = mean(h1[neigh_idx], axis=1)       # [N2, 128]
    out = agg @ W2.T + b2                   # [N2, 41]

Strategy (seed-sharded, gather-free, fp8 features):
  The two gather levels are folded on the host: idx2 = map1[neigh_idx] maps
  every (seed, neighbor) slot directly to a layer-0 node. The host expands
  features to slot order (features[idx2], 25 rows per seed, seed-major),
  quantizes to fp8 e3m4 (x2 scale; relu is positively homogeneous so the
  scale folds into W2), and pre-transposes so the device runs a dense
  fused pass per core:

      featT[:, slot] --5 matmuls (602 = 4x128 + 90), W1 bf16--> PSUM
          --ScalarE relu(+b1)--> h0 bf16 [128, 500]
          --DVE contiguous reduce over each seed's 25 slots--> acc f32r
      acc[128, seeds] --f32r matmul W2--> logitsT --(+b2)--> y [48, seeds]

  Group width 500 = 20 seeds x fan 25, so every group reduces with a
  stride-1 inner dim straight into its acc columns, and matmul cost is
  dominated by per-instruction overhead, so wide groups win.  fp8
  halves feature DMA vs bf16; the matmuls run at bf16 speed (1 col/
  cycle).  Logits for each finished 512-seed segment are emitted inside
  the main loop (no serial epilogue).

  DMA model (measured): each line (contiguous run) is served by one of
  16 engines at ~22.5 GB/s with ~85ns per-descriptor overhead, so
  chunk_time ~ ceil(lines/16) * (line_bytes/22.5G + 85ns).  Features
  are packed so slabs 0-3 of each partition are adjacent in DRAM
  (quad lines of 4*cw bytes, 128+90 lines per chunk instead of 602).
  Chunks ramp 1000/2000/4000 then 6000 so compute starts ~12us in and
  never starves after; ramp chunks zero-pad the 90-row tail slab to 128
  rows so each is a single one-trigger transfer, and chunk 0 rides in
  the SAME uint8 DMA as the w1 weights (bitcast views) — one trigger
  opens the whole pipeline and the LDWEIGHTS dependency is safe by
  construction (single tile, single writer).

  Measured dead ends on this platform (kept for posterity):
  - perf_mode=DoubleRow (and by extension MX) is faster per instruction
    but trips the chip's power throttle (util capped at 50% for half
    the run) for zero net win; the PE multiply rate is the power wall.
    The matmul stream (5 passes x 31250 cols at 1 col/cycle) is the
    ~72us floor.
  - Splitting feature DMA across two trigger-engine queues halves each
    queue's rate (~130 GB/s each vs ~290 single) - shared backend.
  - Per-partition mega-lines (60KB) starve parallelism: only 128 lines
    across 16 engines at 22.5 GB/s each. 8-24KB lines are the sweet
    spot.
  - Dummy p-state warm-up matmuls just delay the real stream.
  - Loading w1t from a non-sync queue races LDWEIGHTS (cross-queue dep
    is only implied by same-queue FIFO order ahead of the first chunk).
  - fp8 h0 storage: no time win (ScalarE/DVE are element-rate bound,
    not byte bound), 2x the output error. bf16 h0 kept.
  Device-side gathers are avoided entirely (SWDGE descriptor generation
  is too slow for ~31k gathered rows/core).

  Sharding: seeds split evenly across 8 cores (1250 seeds each, no
  padding), weights replicated, no collectives. Accuracy: e3m4 features
  + bf16 W1 give rel err ~5.8e-3 (vs 2e-2 budget) — verified
  deterministically in numpy and CoreSim against this problem's fixed
  inputs.  Measured: 89.0-92.1us, median ~90 (bf16 predecessor:
  143.7us); remaining time = 5.8us NEFF init + ~6us first-chunk
  latency + the 72us matmul floor + ~2us stalls/drain.  Ramp-shape
  variants trade start latency against mid-stalls ~1:1 (supply rate
  during ramp ~= demand rate); this shape minimizes the worst case.
"""

import math

import numpy as np
import ml_dtypes

import concourse.bacc as bacc
import concourse.mybir as mybir
import concourse.tile as tile
from concourse import bass_utils

N_CORES = 8
HIDDEN = 128
CPAD = 48  # classes padded 41 -> 48
SF = 2.0        # feature quantization scale
WS = 64.0       # W1 scale for e4m3 (avoids subnormals; power of 2 = exact)
F8 = ml_dtypes.float8_e3m4   # TRN fp8e3 (tail slab)
F8E4 = ml_dtypes.float8_e4m3  # TRN fp8e4 (DoubleRow slabs)
GW = 500        # group width: 20 seeds x fan 25
DR_K = 256      # contraction rows per DoubleRow pass

# Set by test harness: run with trace=True and record exec time here.
TRACE = False
SIM = False
LAST_EXEC_NS = None
LAST_RES = None

_BUILD_CACHE = {}


def _slabs(n_feats):
    slabs = []
    o = 0
    while o + 128 <= n_feats:
        slabs.append((o, 128))
        o += 128
    if o < n_feats:
        slabs.append((o, n_feats - o))
    return slabs


def _chunk_schedule(slots, chunk):
    """chunk0 = GW (rides with w1 for an early compute start); then full
    `chunk`s at peak DMA line size (24KB quad lines); then a geometric
    ramp-DOWN tail. Compute per chunk (~1.62ns/slot) is barely below DMA
    (~1.7ns/slot + fixed), so a big final chunk would serialize ~10us of
    compute after the last DMA byte — the ramp-down nests the compute
    tail inside the DMA tail instead."""
    c0w = 250  # chunk0 rides with w1; small = earlier first matmul
    tail = slots % GW
    rem = slots - tail - c0w
    chunks = [c0w]
    down = [3000, 1500, 750, 500]
    while rem > chunk + sum(down):
        chunks.append(min(chunk, rem - sum(down)))
        rem -= chunks[-1]
    if rem > sum(down):
        chunks.append(rem - sum(down))
        rem = sum(down)
    while down and rem:
        take = min(down.pop(0), rem)
        chunks.append(take)
        rem -= take
    if tail:
        chunks[-1] += tail
    return chunks


def _build(n_feats, nseed, fan, chunk):
    """Build + compile the per-core program (identical on all 8 cores)."""
    F32 = mybir.dt.float32
    F32R = mybir.dt.float32r
    DT_DR = mybir.dt.float8e4   # DoubleRow slabs: features + W1
    DT_TAIL = mybir.dt.float8e3  # tail-slab features
    DT_W = mybir.dt.bfloat16     # tail-slab W1
    DT_H = mybir.dt.bfloat16

    slots = nseed * fan
    assert GW % fan == 0 and chunk % GW == 0

    slabs = _slabs(n_feats)
    ns = len(slabs)
    nfull = sum(1 for _, kk in slabs if kk == 128)
    assert nfull % 2 == 0, "DoubleRow pairing wants an even full-slab count"
    npairs = nfull // 2
    has_tail = ns > nfull

    nc = bacc.Bacc("TRN2", target_bir_lowering=False, debug=False,
                   num_devices=N_CORES)
    chunks = _chunk_schedule(slots, chunk)
    # chunk 0 rides in one DMA with the w1 weights (uint8 tile, bitcast
    # views); later chunks are packed features, ramp ones zero-padded to
    # 128 rows per slab
    # w1 bytes/partition: per DR pass 2 k-tiles x 128 h x 1B, tail 128 h bf16
    W1B = npairs * 2 * HIDDEN + (2 * HIDDEN if has_tail else 0)
    cw0 = chunks[0]
    featT_len = sum(n_feats * cw for cw in chunks[1:])
    featT = nc.dram_tensor("featT", [featT_len], DT_DR,
                           kind="ExternalInput").ap()
    wf0 = nc.dram_tensor("wf0", [128, W1B + ns * cw0], mybir.dt.uint8,
                         kind="ExternalInput").ap()
    w2t = nc.dram_tensor("w2t", [HIDDEN, CPAD], F32R,
                         kind="ExternalInput").ap()
    b1 = nc.dram_tensor("b1", [128, 1], F32, kind="ExternalInput").ap()
    b2 = nc.dram_tensor("b2", [CPAD, 1], F32, kind="ExternalInput").ap()
    # transposed output: yT[c, seed]; host transposes back (tiny)
    y = nc.dram_tensor("y", [CPAD, nseed], F32, kind="ExternalOutput").ap()

    with tile.TileContext(nc) as tc:
        with (
            # f32r accumulator (fp22-rounded, rel 1e-4) feeds the final
            # f32r matmul; quantization noise dominates by 100x
            nc.allow_low_precision(reason="f32r acc for final matmul"),
            tc.tile_pool(name="const", bufs=1) as const,
            tc.tile_pool(name="feat", bufs=4) as featp,
            tc.tile_pool(name="out", bufs=3) as outp,
            tc.tile_pool(name="ph", bufs=4, space="PSUM") as php,
            tc.tile_pool(name="pa", bufs=2, space="PSUM") as pap,
        ):
            # w1 weights + chunk-0 features in ONE transfer on the sync
            # queue (single trigger; LDWEIGHTS and the first matmul both
            # depend on this one tile); other constants via scalar queue
            combo = const.tile([128, W1B + ns * cw0], mybir.dt.uint8)
            nc.sync.dma_start(combo[:], wf0[:])

            def w1_dr_ap(j):
                # [128, 2, 128] e4m3: partition p holds W1T rows
                # (256j + p, 256j + 128 + p) for the two k-tiles
                return combo[:, j * 2 * HIDDEN:(j + 1) * 2 * HIDDEN] \
                    .bitcast(DT_DR).rearrange("p (i h) -> p i h", i=2)

            def w1_tail_ap(kk):
                o = npairs * 2 * HIDDEN
                return combo[:kk, o:o + 2 * HIDDEN].bitcast(DT_W)

            def fk0_dr(j, a, b):
                base = W1B + 2 * j * cw0
                return combo[:, base:base + 2 * cw0].bitcast(DT_DR) \
                    .rearrange("p (s c) -> p s c", s=2)[:, :, a:b]

            def fk0_tail(kk, a, b):
                o = W1B + nfull * cw0
                return combo[:kk, o + a:o + b].bitcast(DT_TAIL)

            b1_sb = const.tile([128, 1], F32)
            nc.scalar.dma_start(b1_sb[:], b1[:])
            w2t_sb = const.tile([128, CPAD], F32R)
            nc.scalar.dma_start(w2t_sb[:], w2t[:])
            b2_sb = const.tile([CPAD, 1], F32)
            nc.scalar.dma_start(b2_sb[:], b2[:])
            # per-seed accumulator over the fan, in h0 space
            acc = const.tile([128, nseed], F32R)

            c0 = 0
            off = 0
            seg0 = 0
            for ci, cw in enumerate(chunks):
                # quad-packed lines: 4 slabs adjacent per partition in
                # DRAM (each DMA line runs on one engine at ~22.5GB/s;
                # fewer, bigger lines win until ~16KB)
                if ci == 0:
                    fk = None
                else:
                    fk = featp.tile([128, ns * cw], DT_DR, tag="fk")
                    step = 4 if nfull % 4 == 0 else 2
                    for j in range(0, nfull, step):
                        nc.sync.dma_start(
                            fk[:, j * cw:(j + step) * cw],
                            featT[off:off + step * 128 * cw].rearrange(
                                "(p c) -> p c", p=128),
                        )
                        off += step * 128 * cw
                    if ns > nfull:
                        kk = slabs[nfull][1]
                        nc.sync.dma_start(
                            fk[:kk, nfull * cw:ns * cw],
                            featT[off:off + kk * cw].rearrange(
                                "(p c) -> p c", p=kk),
                        )
                        off += kk * cw

                for g0 in range(0, cw, GW):
                    gw = min(GW, cw - g0)
                    ph = php.tile([128, GW], F32, tag="ph", space="PSUM")
                    for j in range(npairs):
                        rhs = (fk0_dr(j, g0, g0 + gw) if ci == 0 else
                               fk[:, 2 * j * cw:(2 * j + 2) * cw]
                               .rearrange("p (s c) -> p s c", s=2)
                               [:, :, g0:g0 + gw])
                        nc.tensor.matmul(
                            ph[:, :gw],
                            w1_dr_ap(j),
                            rhs,
                            start=(j == 0),
                            stop=(not has_tail and j == npairs - 1),
                            perf_mode=mybir.MatmulPerfMode.DoubleRow,
                        )
                    if has_tail:
                        kk = slabs[nfull][1]
                        rhs = (fk0_tail(kk, g0, g0 + gw) if ci == 0 else
                               fk[:kk, nfull * cw + g0:nfull * cw + g0 + gw]
                               .bitcast(DT_TAIL))
                        nc.tensor.matmul(
                            ph[:, :gw],
                            w1_tail_ap(kk),
                            rhs,
                            start=False,
                            stop=True,
                        )
                    # relu(+b1) IN PLACE in PSUM, then reduce straight from
                    # PSUM: drops the h0 SBUF tile and its buffer-free
                    # dependency edges (each cost a ~600ns scalar event-sem)
                    nc.scalar.activation(ph[:, :gw], ph[:, :gw],
                                         mybir.ActivationFunctionType.Relu,
                                         bias=b1_sb[:, 0:1])
                    # cols are seed-major (fan inner): one contiguous
                    # reduce per group straight into acc columns
                    s_base = (c0 + g0) // fan
                    nsd = gw // fan
                    nc.vector.reduce_sum(
                        acc[:, s_base:s_base + nsd],
                        ph[:, :gw].rearrange("h (s r) -> h s r", r=fan),
                        axis=mybir.AxisListType.X)
                    # emit logits for every fully-aggregated 512-seed
                    # segment so the epilogue isn't one serial tail.
                    # Trigger 2 groups late: the pa matmul waits on the
                    # segment's last DVE reduce, and the in-order PE queue
                    # would stall behind it if emitted at the boundary.
                    done = s_base + nsd
                    flush = (c0 + g0 + gw == slots)
                    while seg0 < nseed:
                        seg_end = min(seg0 + 512, nseed)
                        if seg_end == nseed and seg_end - seg0 > 64:
                            # split the final segment so only a 64-seed
                            # sliver trails the last group (shorter
                            # reduce->matmul->identity->DMA drain chain)
                            seg_end = nseed - 64
                        if not flush and done < seg_end + 40:
                            break
                        sw = seg_end - seg0
                        pa = pap.tile([CPAD, 512], F32, tag="pa",
                                      space="PSUM")
                        nc.tensor.matmul(pa[:, :sw], w2t_sb[:],
                                         acc[:, seg0:seg0 + sw],
                                         start=True, stop=True)
                        yo = outp.tile([CPAD, 512], F32, tag="yo")
                        nc.scalar.activation(
                            yo[:, :sw], pa[:, :sw],
                            mybir.ActivationFunctionType.Identity,
                            bias=b2_sb[:, 0:1])
                        nc.sync.dma_start(y[:, seg0:seg0 + sw],
                                          yo[:, :sw])
                        seg0 += sw
                c0 += cw

    nc.compile()
    return nc


def kernel(features, W1, b1, W2, b2, map1, neigh_idx):
    global LAST_EXEC_NS, LAST_RES
    features = np.asarray(features, dtype=np.float32)
    W1 = np.asarray(W1, dtype=np.float32)
    b1 = np.asarray(b1, dtype=np.float32)
    W2 = np.asarray(W2, dtype=np.float32)
    b2 = np.asarray(b2, dtype=np.float32)
    map1 = np.asarray(map1).astype(np.int64)
    neigh_idx = np.asarray(neigh_idx).astype(np.int64)

    n0, n_feats = features.shape
    hidden = W1.shape[0]
    classes = W2.shape[0]
    n2, fan = neigh_idx.shape
    assert hidden == HIDDEN and classes <= CPAD

    idx2 = map1[neigh_idx]  # [N2, fan] -> layer-0 node per slot

    # split seeds evenly; pad only to a multiple of N_CORES
    spc = math.ceil(n2 / N_CORES)  # seeds per core
    n2_pad = spc * N_CORES
    if n2_pad > n2:
        idx2 = np.concatenate(
            [idx2, np.zeros((n2_pad - n2, fan), dtype=idx2.dtype)], axis=0)

    chunk = 6000
    nc = _get_built(n_feats, spc, fan, chunk)
    slots = spc * fan
    chunks = _chunk_schedule(slots, chunk)
    slabs = _slabs(n_feats)

    # quantize once, gather bytes per slot (cheap on host).
    # DR slabs (features rows [0, 128*nfull)): e4m3 * SF; tail rows: e3m4 * SF
    nfull_h = sum(1 for _, kk in slabs if kk == 128)
    npairs_h = nfull_h // 2
    kdr = 128 * nfull_h
    fq = np.empty((n0, n_feats), dtype=np.uint8)
    fq[:, :kdr] = np.asarray(features[:, :kdr] * SF, F8E4).view(np.uint8)
    fq[:, kdr:] = np.asarray(features[:, kdr:] * SF, F8).view(np.uint8)
    # W1 scaled by WS (exact power of two): DR passes e4m3, tail bf16
    w1s = (W1.T * WS).astype(np.float32)              # [F, 128]
    W1B = npairs_h * 2 * HIDDEN + (2 * HIDDEN if n_feats > kdr else 0)
    w1q_u8 = np.zeros((128, W1B), dtype=np.uint8)
    for j in range(npairs_h):
        for i in range(2):
            o = 256 * j + 128 * i
            blk = np.asarray(w1s[o:o + 128], F8E4)    # [128 part, 128 h]
            w1q_u8[:, (2 * j + i) * HIDDEN:(2 * j + i + 1) * HIDDEN] = \
                blk.view(np.uint8)
    if n_feats > kdr:
        tl = np.zeros((128, HIDDEN), dtype=np.float32)
        tl[:n_feats - kdr] = w1s[kdr:]
        w1q_u8[:, npairs_h * 2 * HIDDEN:] = \
            tl.astype(ml_dtypes.bfloat16).view(np.uint8)
    b1_in = np.ascontiguousarray((b1 * SF * WS).reshape(HIDDEN, 1)).astype(
        np.float32)
    w2t = np.zeros((HIDDEN, CPAD), dtype=np.float32)
    w2t[:, :classes] = (W2 / (SF * WS * fan)).T
    b2_in = np.zeros((CPAD, 1), dtype=np.float32)
    b2_in[:classes, 0] = b2

    in_maps = []
    for c in range(N_CORES):
        slot_ids = idx2[c * spc:(c + 1) * spc].ravel()  # seed-major
        featT = np.ascontiguousarray(fq[slot_ids].T)  # [F, slots] fp8 bytes
        # pack slab-PAIR-interleaved per chunk: partition p's rows for
        # slabs (2j, 2j+1) are adjacent -> 2*cw-byte DMA lines
        parts = []
        c0 = 0
        wf0 = None
        for ci, cw in enumerate(chunks):
            if ci == 0:
                ns_h = len(slabs)
                blk = np.zeros((128, ns_h, cw), dtype=featT.dtype)
                for i, (o, kk) in enumerate(slabs):
                    blk[:kk, i] = featT[o:o + kk, c0:c0 + cw]
                wf0 = np.concatenate(
                    [w1q_u8, blk.reshape(128, -1).view(np.uint8)], axis=1)
            else:
                step = 4 if nfull_h % 4 == 0 else 2
                for j in range(0, nfull_h, step):
                    blk = featT[j * 128:(j + step) * 128, c0:c0 + cw]
                    parts.append(
                        blk.reshape(step, 128, cw).transpose(1, 0, 2)
                        .ravel())
                if len(slabs) > nfull_h:
                    o, kk = slabs[nfull_h]
                    parts.append(featT[o:o + kk, c0:c0 + cw].ravel())
            c0 += cw
        featT_packed = np.concatenate(parts).view(F8E4)
        in_maps.append({
            "featT": featT_packed,
            "wf0": wf0,
            "w2t": w2t,
            "b1": b1_in,
            "b2": b2_in,
        })

    if SIM:
        from concourse.bass_interp import CoreSim

        ys = []
        for c in range(N_CORES if SIM is True else int(SIM)):
            sim = CoreSim(nc, trace=False)
            for k, v in in_maps[c].items():
                sim.tensor(k)[:] = v
            sim.simulate(check_with_hw=False)
            ys.append(sim.tensor("y").T.copy())
        LAST_EXEC_NS = None
        yf = np.concatenate(ys, axis=0)
        if len(ys) < N_CORES:  # partial sim: pad to full shape with zeros
            yf = np.concatenate(
                [yf, np.zeros((n2_pad - yf.shape[0], CPAD), yf.dtype)], axis=0)
    else:
        res = bass_utils.run_bass_kernel_spmd(
            nc, in_maps, core_ids=list(range(N_CORES)), trace=TRACE)
        LAST_EXEC_NS = res.exec_time_ns
        LAST_RES = res
        yf = np.concatenate(
            [res.results[c]["y"].T for c in range(N_CORES)], axis=0)
    return np.ascontiguousarray(yf[:n2, :classes]).astype(np.float32)


def _get_built(n_feats, nseed, fan, chunk):
    key = (n_feats, nseed, fan, chunk)
    if key not in _BUILD_CACHE:
        _BUILD_CACHE[key] = _build(n_feats, nseed, fan, chunk)
    return _BUILD_CACHE[key]

